# revision 1
# baseline (speedup 1.0000x reference)
"""Trainium2 Bass kernel for nn_AttentionMemory (sparse_attention).

Reference computation (per batch b):
    mk = Mk[b].reshape(CK, N); qk = Qk[b].reshape(CK, N)
    affinity[m, q] = softmax_m( (2*mk[:,m]@qk[:,q] - |mk[:,m]|^2) / sqrt(CK) )

Sharding: 8 cores = 4 batches x 2 query-halves. Each core computes the full
memory (softmax) axis for 2048 of one batch's queries — no collectives.

Per-core layout: queries on partitions (16 q-tiles of 128), memory positions
on the free axis. Per q-tile, the 4096-wide pre-softmax row is built one PSUM
bank (512 memory columns) at a time:
    psum_bank = matmul(-0.5*ones, mk*mk) accum matmul(qk_tile, mk)
ScalarE applies exp(2/sqrt(CK)*psum) per bank with a fused free-axis row-sum
(accum_out); VectorE adds the 8 partial sums (tree), takes the reciprocal and
applies it as a per-partition tensor_scalar multiply; DMA writes [128, 2048]
half-rows. The logits are bounded (~[-30, +8]) so no max-subtraction pass is
needed. The host transposes [q, m] -> [m, q] while gathering.

Implementation notes:
  * fp32r (TF32) matmuls keep the PE at 1 cycle/column (fp32 is 4x slower);
    fp32r operands must be produced by compute engines (rounding rule), so
    inputs are funneled through VectorE copies/multiplies. Stationary tiles
    are materialized contiguously per q-tile.
  * Walrus caps instructions at one sync wait. After Tile scheduling, any
    instruction with N>1 waits has N-1 of them spilled onto single-wait Drain
    instructions inserted before it on the same engine — semantically
    equivalent (waits are an AND over monotonic semaphores, executed in order
    on the same sequencer).
"""
import math
import numpy as np

import bass_rust
from concourse import bass, tile, mybir
from concourse.bass_utils import run_bass_kernel_spmd

B, CK, HH, WW = 4, 128, 64, 64
N = HH * WW            # 4096 memory positions / queries per batch
QH = N // 2            # 2048 queries per core
N_CORES = 8
QTILE = 128            # queries per q-tile (PSUM partition dim)
MCHUNK = 512           # memory cols per matmul / exp (one PSUM bank, fp32)
MHALF = 2048           # memory cols per output DMA block
SCALE = 2.0 / math.sqrt(CK)
F32 = mybir.dt.float32
F32R = mybir.dt.float32r


def _build():
    nc = bass.Bass("TRN2", target_bir_lowering=False, debug=False,
                   num_devices=N_CORES)
    mk_d = nc.dram_tensor("mk", [CK, N], F32, kind="ExternalInput").ap()
    qk_d = nc.dram_tensor("qk", [CK, QH], F32, kind="ExternalInput").ap()
    out_d = nc.dram_tensor("out", [QH, N], F32, kind="ExternalOutput").ap()

    n_qt = QH // QTILE          # 16
    with tile.TileContext(nc) as tc:
        with tc.tile_pool(name="inp", bufs=1) as inp_pool, \
             tc.tile_pool(name="exp", bufs=2) as exp_pool, \
             tc.tile_pool(name="outb", bufs=4) as out_pool, \
             tc.tile_pool(name="small", bufs=4) as small_pool, \
             tc.tile_pool(name="psum", bufs=4, space="PSUM") as psum_pool:

            mk_sb = inp_pool.tile([CK, N], F32, tag="mkraw")
            qk_sb = inp_pool.tile([CK, QH], F32, tag="qkraw")
            nc.sync.dma_start(out=mk_sb[:], in_=mk_d)
            nc.sync.dma_start(out=qk_sb[:], in_=qk_d)

            # All matmul operands produced on DVE (fp32r rounding rule) and
            # contiguous: per-q-tile stationary copies of qk.
            mk2 = inp_pool.tile([CK, N], F32R, tag="mk2")
            mksq = inp_pool.tile([CK, N], F32R, tag="mksq")
            mhalf_raw = inp_pool.tile([128, 128], F32, tag="mhalfraw")
            mhalf = inp_pool.tile([128, 128], F32R, tag="mhalf")
            nc.vector.tensor_copy(mk2[:], mk_sb[:])
            nc.vector.tensor_mul(mksq[:], mk_sb[:], mk_sb[:])
            nc.vector.memset(mhalf_raw[:], -0.5)
            nc.vector.tensor_copy(mhalf[:], mhalf_raw[:])
            qk_tiles = []
            for t in range(n_qt):
                qt = inp_pool.tile([CK, QTILE], F32R, tag=f"qkt{t}")
                nc.vector.tensor_copy(
                    qt[:], qk_sb[:, t * QTILE:(t + 1) * QTILE])
                qk_tiles.append(qt)

            for t in range(n_qt):
                exp_t = exp_pool.tile([QTILE, N], F32, tag="exp")
                parts = small_pool.tile([QTILE, 8], F32, tag="parts")
                for j in range(8):
                    m0 = j * MCHUNK
                    ps = psum_pool.tile([QTILE, MCHUNK], F32, tag="ps")
                    nc.tensor.matmul(ps[:], mhalf[:],
                                     mksq[:, m0:m0 + MCHUNK],
                                     start=True, stop=False)
                    nc.tensor.matmul(ps[:], qk_tiles[t],
                                     mk2[:, m0:m0 + MCHUNK],
                                     start=False, stop=True)
                    nc.scalar.activation(
                        exp_t[:, m0:m0 + MCHUNK], ps[:],
                        mybir.ActivationFunctionType.Exp,
                        scale=SCALE, accum_out=parts[:, j:j + 1])
                # denominator: tree-add the 8 partials, then reciprocal
                s4 = small_pool.tile([QTILE, 4], F32, tag="s4")
                s2 = small_pool.tile([QTILE, 2], F32, tag="s2")
                s1 = small_pool.tile([QTILE, 1], F32, tag="s1")
                rec_t = small_pool.tile([QTILE, 1], F32, tag="rec")
                nc.vector.tensor_add(s4[:], parts[:, 0:4], parts[:, 4:8])
                nc.vector.tensor_add(s2[:], s4[:, 0:2], s4[:, 2:4])
                nc.vector.tensor_add(s1[:], s2[:, 0:1], s2[:, 1:2])
                nc.vector.reciprocal(rec_t[:], s1[:])
                for h in range(2):
                    o = out_pool.tile([QTILE, MHALF], F32, tag="o")
                    nc.vector.tensor_scalar_mul(
                        o[:], exp_t[:, h * MHALF:(h + 1) * MHALF], rec_t[:])
                    nc.sync.dma_start(
                        out=out_d[t * QTILE:(t + 1) * QTILE,
                                  h * MHALF:(h + 1) * MHALF],
                        in_=o[:])
    _strip_self_waits(nc)
    return nc


def _strip_self_waits(nc):
    """Walrus rejects instructions carrying more than one sync wait.

    Conservative fix: for any instruction with N>1 waits, keep the last wait
    on the instruction and spill the other N-1 onto single-wait Drain
    instructions inserted immediately before it on the same engine. All waits
    still execute, in program order, on the same sequencer; semaphores are
    monotonic so splitting an AND of waits into a sequence is equivalent.
    """
    for fn in nc.m.functions:
        for blk in fn.blocks:
            il = blk.instructions
            new_il = []
            changed = False
            for ins in il:
                si = getattr(ins, "sync_info", None)
                if si is not None and len(si.on_wait) > 1:
                    changed = True
                    waits = list(si.on_wait)
                    for k, w in enumerate(waits[:-1]):
                        d = mybir.InstDrain(
                            name=f"{ins.name}_w{k}",
                            ins=[], outs=[], bass_is_fusable=False)
                        d.engine = ins.engine
                        d.sync_info = bass_rust.SyncInfo(on_wait=[w],
                                                         on_update=[])
                        new_il.append(d)
                    ins.sync_info = bass_rust.SyncInfo(on_wait=[waits[-1]],
                                                      on_update=si.on_update)
                new_il.append(ins)
            if changed:
                blk.instructions = new_il


_NC_CACHE = None


def kernel(Mk: np.ndarray, Qk: np.ndarray) -> np.ndarray:
    global _NC_CACHE
    if _NC_CACHE is None:
        _NC_CACHE = _build()
    nc = _NC_CACHE

    Mk = np.ascontiguousarray(np.asarray(Mk), dtype=np.float32)
    Qk = np.ascontiguousarray(np.asarray(Qk), dtype=np.float32)

    in_maps = []
    for c in range(N_CORES):
        b, half = c // 2, c % 2
        mk = Mk[b].reshape(CK, N)
        qk = np.ascontiguousarray(Qk[b].reshape(CK, N)[:, half * QH:(half + 1) * QH])
        in_maps.append({"mk": mk, "qk": qk})

    res = run_bass_kernel_spmd(nc, in_maps, core_ids=list(range(N_CORES)))

    out = np.empty((B, N, N), dtype=np.float32)
    for c in range(N_CORES):
        b, half = c // 2, c % 2
        out[b, :, half * QH:(half + 1) * QH] = res.results[c]["out"].T
    return out



# revision 2
# speedup vs baseline: 1.2932x; 1.2932x over previous
"""Trainium2 Bass kernel for nn_AttentionMemory (sparse_attention).

Reference computation (per batch b):
    mk = Mk[b].reshape(CK, N); qk = Qk[b].reshape(CK, N)
    affinity[m, q] = softmax_m( (2*mk[:,m]@qk[:,q] - |mk[:,m]|^2) / sqrt(CK) )

Sharding: 8 cores = 4 batches x 2 query-halves. Each core computes the full
memory (softmax) axis for 2048 of one batch's queries — no collectives.

Per-core layout: queries on partitions (16 q-tiles of 128), memory positions
on the free axis. Per q-tile, the 4096-wide pre-softmax row is built in two
2048-column halves, each a 4-bank PSUM tile (double-buffered ping-pong):
    for j in 0..4: psum[:, j*512:] = matmul(-0.5*ones, mksq)   (start)
    for j in 0..4: psum[:, j*512:] += matmul(qk_tile, mk)      (stop)
ScalarE applies exp(2/sqrt(CK)*psum) over the whole 2048-wide half in ONE
activation (PSUM reads may span banks; only matmul writes are bank-limited),
writing bf16 numerators with a fused fp32 row-sum (accum_out). VectorE adds
the 2 partials, takes the reciprocal and applies it as a per-partition
tensor_scalar multiply at 4x bf16 rate; DMA writes bf16 [128, 4096] rows.
The host transposes [q, m] -> [m, q] and casts bf16 -> fp32 while gathering.

Implementation notes:
  * Matmul operands are fp16: measured HW runs fp32r matmuls at ~2 cycles/
    column (486ns median for N=512) while fp16/bf16 stream at 1 cycle/column.
    fp16 keeps 10 mantissa bits: end-to-end rel err ~3e-3 (numpy-simulated)
    vs the 2e-2 gate. PSUM accumulation stays fp32.
  * Within a half, the 4 "-0.5*|mk|^2" matmuls share one stationary operand
    and run back-to-back, then the 4 "qk@mk" matmuls share the q-tile
    stationary: 2 LDWEIGHTS per half instead of 8.
  * bf16 numerators halve DVE normalize time (4x mode) and halve the output
    DMA bytes (16.8MB/core); the logits are bounded (~[-30, +8]) so no
    max-subtraction pass is needed.
  * Walrus caps instructions at one sync wait. After Tile scheduling, any
    instruction with N>1 waits has N-1 of them spilled onto single-wait Drain
    instructions inserted before it on the same engine — semantically
    equivalent (waits are an AND over monotonic semaphores, executed in order
    on the same sequencer).
"""
import math
import numpy as np

import bass_rust
from concourse import bass, tile, mybir
from concourse.bass_utils import run_bass_kernel_spmd

B, CK, HH, WW = 4, 128, 64, 64
N = HH * WW            # 4096 memory positions / queries per batch
QH = N // 2            # 2048 queries per core
N_CORES = 8
QTILE = 128            # queries per q-tile (PSUM partition dim)
MCHUNK = 512           # memory cols per matmul (one PSUM bank, fp32)
MHALF = 2048           # memory cols per PSUM tile / activation / DMA block
SCALE = 2.0 / math.sqrt(CK)
F32 = mybir.dt.float32
F16 = mybir.dt.float16
BF16 = mybir.dt.bfloat16


def _build():
    nc = bass.Bass("TRN2", target_bir_lowering=False, debug=False,
                   num_devices=N_CORES)
    mk_d = nc.dram_tensor("mk", [CK, N], F32, kind="ExternalInput").ap()
    qk_d = nc.dram_tensor("qk", [CK, QH], F32, kind="ExternalInput").ap()
    out_d = nc.dram_tensor("out", [QH, N], BF16, kind="ExternalOutput").ap()

    n_qt = QH // QTILE          # 16
    with tile.TileContext(nc) as tc:
        with tc.tile_pool(name="inp", bufs=1) as inp_pool, \
             tc.tile_pool(name="exp", bufs=2) as exp_pool, \
             tc.tile_pool(name="outb", bufs=2) as out_pool, \
             tc.tile_pool(name="small", bufs=4) as small_pool, \
             tc.tile_pool(name="psum", bufs=2, space="PSUM") as psum_pool:

            qk_sb = inp_pool.tile([CK, QH], F32, tag="qkraw")
            mk_sb = inp_pool.tile([CK, N], F32, tag="mkraw")
            nc.sync.dma_start(out=qk_sb[:], in_=qk_d)
            nc.sync.dma_start(out=mk_sb[:], in_=mk_d)

            # fp16 matmul operands, produced on DVE. Stationary tiles are
            # materialized contiguously per q-tile.
            mhalf = inp_pool.tile([128, 128], F16, tag="mhalf")
            nc.vector.memset(mhalf[:], -0.5)
            qk_tiles = []
            for t in range(n_qt):
                qt = inp_pool.tile([CK, QTILE], F16, tag=f"qkt{t}")
                nc.vector.tensor_copy(
                    qt[:], qk_sb[:, t * QTILE:(t + 1) * QTILE])
                qk_tiles.append(qt)
            mk_h = inp_pool.tile([CK, N], F16, tag="mkh")
            mksq_h = inp_pool.tile([CK, N], F16, tag="mksqh")
            nc.vector.tensor_copy(mk_h[:], mk_sb[:])
            nc.vector.tensor_mul(mksq_h[:], mk_sb[:], mk_sb[:])

            for t in range(n_qt):
                exp_t = exp_pool.tile([QTILE, N], BF16, tag="exp")
                parts = small_pool.tile([QTILE, 2], F32, tag="parts")
                for h in range(2):
                    m0 = h * MHALF
                    ps = psum_pool.tile([QTILE, MHALF], F32, tag="ps")
                    for j in range(4):
                        c0 = j * MCHUNK
                        nc.tensor.matmul(ps[:, c0:c0 + MCHUNK], mhalf[:],
                                         mksq_h[:, m0 + c0:m0 + c0 + MCHUNK],
                                         start=True, stop=False)
                    for j in range(4):
                        c0 = j * MCHUNK
                        nc.tensor.matmul(ps[:, c0:c0 + MCHUNK], qk_tiles[t],
                                         mk_h[:, m0 + c0:m0 + c0 + MCHUNK],
                                         start=False, stop=True)
                    nc.scalar.activation(
                        exp_t[:, m0:m0 + MHALF], ps[:],
                        mybir.ActivationFunctionType.Exp,
                        scale=SCALE, accum_out=parts[:, h:h + 1])
                # denominator: add the 2 partials, then reciprocal
                s1 = small_pool.tile([QTILE, 1], F32, tag="s1")
                rec_t = small_pool.tile([QTILE, 1], F32, tag="rec")
                nc.vector.tensor_add(s1[:], parts[:, 0:1], parts[:, 1:2])
                nc.vector.reciprocal(rec_t[:], s1[:])
                o = out_pool.tile([QTILE, N], BF16, tag="o")
                nc.vector.tensor_scalar_mul(o[:], exp_t[:], rec_t[:])
                nc.sync.dma_start(
                    out=out_d[t * QTILE:(t + 1) * QTILE, :], in_=o[:])
    _strip_self_waits(nc)
    return nc


def _strip_self_waits(nc):
    """Walrus rejects instructions carrying more than one sync wait.

    Conservative fix: for any instruction with N>1 waits, keep the last wait
    on the instruction and spill the other N-1 onto single-wait Drain
    instructions inserted immediately before it on the same engine. All waits
    still execute, in program order, on the same sequencer; semaphores are
    monotonic so splitting an AND of waits into a sequence is equivalent.
    """
    for fn in nc.m.functions:
        for blk in fn.blocks:
            il = blk.instructions
            new_il = []
            changed = False
            for ins in il:
                si = getattr(ins, "sync_info", None)
                if si is not None and len(si.on_wait) > 1:
                    changed = True
                    waits = list(si.on_wait)
                    for k, w in enumerate(waits[:-1]):
                        d = mybir.InstDrain(
                            name=f"{ins.name}_w{k}",
                            ins=[], outs=[], bass_is_fusable=False)
                        d.engine = ins.engine
                        d.sync_info = bass_rust.SyncInfo(on_wait=[w],
                                                         on_update=[])
                        new_il.append(d)
                    ins.sync_info = bass_rust.SyncInfo(on_wait=[waits[-1]],
                                                      on_update=si.on_update)
                new_il.append(ins)
            if changed:
                blk.instructions = new_il


_NC_CACHE = None


def kernel(Mk: np.ndarray, Qk: np.ndarray) -> np.ndarray:
    global _NC_CACHE
    if _NC_CACHE is None:
        _NC_CACHE = _build()
    nc = _NC_CACHE

    Mk = np.ascontiguousarray(np.asarray(Mk), dtype=np.float32)
    Qk = np.ascontiguousarray(np.asarray(Qk), dtype=np.float32)

    in_maps = []
    for c in range(N_CORES):
        b, half = c // 2, c % 2
        mk = Mk[b].reshape(CK, N)
        qk = np.ascontiguousarray(Qk[b].reshape(CK, N)[:, half * QH:(half + 1) * QH])
        in_maps.append({"mk": mk, "qk": qk})

    res = run_bass_kernel_spmd(nc, in_maps, core_ids=list(range(N_CORES)))

    out = np.empty((B, N, N), dtype=np.float32)
    for c in range(N_CORES):
        b, half = c // 2, c % 2
        out[b, :, half * QH:(half + 1) * QH] = \
            np.asarray(res.results[c]["out"]).astype(np.float32).T
    return out


# revision 7
# speedup vs baseline: 1.3522x; 1.0457x over previous
"""Trainium2 Bass kernel for nn_AttentionMemory (sparse_attention).

Reference computation (per batch b):
    mk = Mk[b].reshape(CK, N); qk = Qk[b].reshape(CK, N)
    affinity[m, q] = softmax_m( (2*mk[:,m]@qk[:,q] - |mk[:,m]|^2) / sqrt(CK) )

Sharding: 8 cores = 4 batches x 2 query-halves. Each core computes the full
memory (softmax) axis for 2048 of one batch's queries — no collectives.

Per-core layout: queries on partitions (16 q-tiles of 128), memory positions
on the free axis. Per q-tile, the 4096-wide pre-softmax row is built in two
2048-column halves, each a 4-bank PSUM tile (double-buffered ping-pong):
    for j in 0..4: psum[:, j*512:] = matmul(qk_tile, mk)        (start)
    for j in 0..4: psum[:, j*512:] += matmul(-0.5*ones, mksq)   (stop)
ScalarE applies exp(2/sqrt(CK)*psum) over the whole 2048-wide half in ONE
activation (PSUM reads may span banks; only matmul writes are bank-limited),
writing bf16 numerators with a fused fp32 row-sum (accum_out). VectorE adds
the 2 partials, takes the reciprocal and applies it as a per-partition
tensor_scalar multiply at 4x bf16 rate; DMA writes bf16 [128, 4096] rows.
The host transposes [q, m] -> [m, q] and casts bf16 -> fp32 while gathering.

Implementation notes:
  * Matmul operands are fp16: measured HW runs fp32r matmuls at ~2 cycles/
    column while fp16/bf16 stream at 1 cycle/column. fp16 keeps 10 mantissa
    bits: end-to-end rel err ~3e-3 (numpy-simulated) vs the 2e-2 gate. PSUM
    accumulation stays fp32.
  * Within a half, the 4 "qk@mk" matmuls share the q-tile stationary and the
    4 "-0.5*|mk|^2" matmuls share the ones stationary; the
    _elide_redundant_ldweights BIR pass drops the repeated LDWEIGHTS (bass
    emits one per matmul) so same-weight matmuls issue back-to-back.
  * Inputs are DMA'd and cast in 1024-column chunks so the first matmuls
    start after ~1/4 of the mk bytes land instead of after the full 3MB
    load + full-width casts (the v1 kernel spent ~16us idle at the head).
  * bf16 numerators halve DVE normalize time (4x mode) and halve the output
    DMA bytes (16.8MB/core); the logits are bounded (~[-30, +8]) so no
    max-subtraction pass is needed.
  * Walrus caps instructions at one sync wait. After Tile scheduling, any
    instruction with N>1 waits has N-1 of them spilled onto single-wait Drain
    instructions inserted before it on the same engine — semantically
    equivalent (waits are an AND over monotonic semaphores, executed in order
    on the same sequencer).
"""
import math
import numpy as np

import bass_rust
from concourse import bass, tile, mybir
from concourse.bass_utils import run_bass_kernel_spmd

B, CK, HH, WW = 4, 128, 64, 64
N = HH * WW            # 4096 memory positions / queries per batch
QH = N // 2            # 2048 queries per core
N_CORES = 8
QTILE = 128            # queries per q-tile (PSUM partition dim)
MCHUNK = 512           # memory cols per matmul (one PSUM bank, fp32)
MHALF = 2048           # memory cols per PSUM tile / activation
ICHUNK = 1024          # input DMA / cast chunk width
SCALE = 2.0 / math.sqrt(CK)
F32 = mybir.dt.float32
F16 = mybir.dt.float16
BF16 = mybir.dt.bfloat16


def _elide_redundant_ldweights(nc):
    """Remove InstLdweights that reload the weights already resident in the
    PE array (same tensor, offset, access pattern, dtype and mode as the
    previous load, with only matmuls in between). bass emits one LDWEIGHTS
    per matmul; within a 4-matmul same-stationary group the last 3 are
    redundant, and on HW the per-matmul reload forces a full array drain
    before each fill (measured 379ns + 117ns gap per 512-column fp16 matmul
    vs the ~213ns streaming rate). A redundant load that carries semaphore
    waits/updates is converted to a Drain with identical sync_info so the
    synchronization graph is unchanged; one with no sync info is dropped.
    Standalone-LDWEIGHTS + non-self-loading matmul is only correct for
    16-bit weights (fp32/fp32r break in walrus) — all matmuls here are fp16.
    """
    def key(ins):
        w = ins.ins[0]
        return (
            getattr(w, "memref", None), getattr(w, "offset", None),
            str(getattr(w, "ap", None)), str(getattr(w, "dtype", None)),
            str(getattr(ins, "perf_mode", None)),
            str(getattr(ins, "is_transpose", None)),
            str(getattr(ins, "tile_position", None)),
        )

    for fn in nc.m.functions:
        for blk in fn.blocks:
            last_key = None
            new_il = []
            for ins in blk.instructions:
                tn = type(ins).__name__
                if tn == "InstLdweights":
                    k = key(ins)
                    if k == last_key:
                        si = getattr(ins, "sync_info", None)
                        if si is not None and (si.on_wait or si.on_update):
                            d = mybir.InstDrain(
                                name=f"{ins.name}_ldwskip",
                                ins=[], outs=[], bass_is_fusable=False)
                            d.engine = ins.engine
                            d.sync_info = si
                            new_il.append(d)
                        continue
                    last_key = k
                elif tn == "InstMatmult":
                    pass  # matmuls leave the loaded weights untouched
                elif getattr(ins, "engine", None) == getattr(
                        nc.tensor, "engine", None):
                    # any other PE-queue instruction: be conservative
                    last_key = None
                new_il.append(ins)
            blk.instructions = new_il


def _build():
    nc = bass.Bass("TRN2", target_bir_lowering=False, debug=False,
                   num_devices=N_CORES)
    mk_d = nc.dram_tensor("mk", [CK, N], F32, kind="ExternalInput").ap()
    qk_d = nc.dram_tensor("qk", [CK, QH], F32, kind="ExternalInput").ap()
    out_d = nc.dram_tensor("out", [QH, N], BF16, kind="ExternalOutput").ap()

    n_qt = QH // QTILE          # 16
    n_mc = N // ICHUNK          # 4 mk chunks
    n_qc = QH // ICHUNK         # 2 qk chunks
    with tile.TileContext(nc) as tc:
        with tc.tile_pool(name="inp", bufs=1) as inp_pool, \
             tc.tile_pool(name="exp", bufs=2) as exp_pool, \
             tc.tile_pool(name="outb", bufs=2) as out_pool, \
             tc.tile_pool(name="small", bufs=4) as small_pool, \
             tc.tile_pool(name="psum", bufs=2, space="PSUM") as psum_pool:

            # Chunked input load + fp16 operand prep, interleaved so the
            # first q-tile's matmuls only wait on chunk 0 of each input.
            qkc = [inp_pool.tile([CK, ICHUNK], F32, name=f"qkc{c}", tag=f"qkc{c}")
                   for c in range(n_qc)]
            mkc = [inp_pool.tile([CK, ICHUNK], F32, name=f"mkc{c}", tag=f"mkc{c}")
                   for c in range(n_mc)]
            mk_h = [inp_pool.tile([CK, ICHUNK], F16, name=f"mkh{c}", tag=f"mkh{c}")
                    for c in range(n_mc)]
            mksq_h = [inp_pool.tile([CK, ICHUNK], F16, name=f"mksqh{c}", tag=f"mksqh{c}")
                      for c in range(n_mc)]
            mhalf = inp_pool.tile([128, 128], F16, tag="mhalf")
            qk_tiles = [inp_pool.tile([CK, QTILE], F16, name=f"qkt{t}", tag=f"qkt{t}")
                        for t in range(n_qt)]

            def prep_qk_chunk(c):
                nc.sync.dma_start(out=qkc[c][:],
                                  in_=qk_d[:, c * ICHUNK:(c + 1) * ICHUNK])
                for t in range(c * 8, c * 8 + 8):
                    off = t * QTILE - c * ICHUNK
                    nc.vector.tensor_copy(qk_tiles[t][:],
                                          qkc[c][:, off:off + QTILE])

            def prep_mk_chunk(c):
                nc.sync.dma_start(out=mkc[c][:],
                                  in_=mk_d[:, c * ICHUNK:(c + 1) * ICHUNK])
                nc.vector.tensor_copy(mk_h[c][:], mkc[c][:])
                nc.vector.tensor_mul(mksq_h[c][:], mkc[c][:], mkc[c][:])

            prep_qk_chunk(0)
            nc.vector.memset(mhalf[:], -0.5)
            prep_mk_chunk(0)
            prep_mk_chunk(1)
            prep_qk_chunk(1)
            prep_mk_chunk(2)
            prep_mk_chunk(3)

            for t in range(n_qt):
                exp_t = exp_pool.tile([QTILE, N], BF16, tag="exp")
                parts = small_pool.tile([QTILE, 2], F32, tag="parts")
                for h in range(2):
                    ps = psum_pool.tile([QTILE, MHALF], F32, tag="ps")
                    for j in range(4):
                        g = h * MHALF + j * MCHUNK
                        c, off = divmod(g, ICHUNK)
                        nc.tensor.matmul(
                            ps[:, j * MCHUNK:(j + 1) * MCHUNK], qk_tiles[t],
                            mk_h[c][:, off:off + MCHUNK],
                            start=True, stop=False)
                    for j in range(4):
                        g = h * MHALF + j * MCHUNK
                        c, off = divmod(g, ICHUNK)
                        nc.tensor.matmul(
                            ps[:, j * MCHUNK:(j + 1) * MCHUNK], mhalf[:],
                            mksq_h[c][:, off:off + MCHUNK],
                            start=False, stop=True)
                    nc.scalar.activation(
                        exp_t[:, h * MHALF:(h + 1) * MHALF], ps[:],
                        mybir.ActivationFunctionType.Exp,
                        scale=SCALE, accum_out=parts[:, h:h + 1])
                # denominator: add the 2 partials, then reciprocal
                s1 = small_pool.tile([QTILE, 1], F32, tag="s1")
                rec_t = small_pool.tile([QTILE, 1], F32, tag="rec")
                nc.vector.tensor_add(s1[:], parts[:, 0:1], parts[:, 1:2])
                nc.vector.reciprocal(rec_t[:], s1[:])
                o = out_pool.tile([QTILE, N], BF16, tag="o")
                nc.vector.tensor_scalar_mul(o[:], exp_t[:], rec_t[:])
                nc.sync.dma_start(
                    out=out_d[t * QTILE:(t + 1) * QTILE, :], in_=o[:])
    _elide_redundant_ldweights(nc)
    _strip_self_waits(nc)
    return nc


def _strip_self_waits(nc):
    """Walrus rejects instructions carrying more than one sync wait.

    Conservative fix: for any instruction with N>1 waits, keep the last wait
    on the instruction and spill the other N-1 onto single-wait Drain
    instructions inserted immediately before it on the same engine. All waits
    still execute, in program order, on the same sequencer; semaphores are
    monotonic so splitting an AND of waits into a sequence is equivalent.
    """
    for fn in nc.m.functions:
        for blk in fn.blocks:
            il = blk.instructions
            new_il = []
            changed = False
            for ins in il:
                si = getattr(ins, "sync_info", None)
                if si is not None and len(si.on_wait) > 1:
                    changed = True
                    waits = list(si.on_wait)
                    for k, w in enumerate(waits[:-1]):
                        d = mybir.InstDrain(
                            name=f"{ins.name}_w{k}",
                            ins=[], outs=[], bass_is_fusable=False)
                        d.engine = ins.engine
                        d.sync_info = bass_rust.SyncInfo(on_wait=[w],
                                                         on_update=[])
                        new_il.append(d)
                    ins.sync_info = bass_rust.SyncInfo(on_wait=[waits[-1]],
                                                      on_update=si.on_update)
                new_il.append(ins)
            if changed:
                blk.instructions = new_il


_NC_CACHE = None


def kernel(Mk: np.ndarray, Qk: np.ndarray) -> np.ndarray:
    global _NC_CACHE
    if _NC_CACHE is None:
        _NC_CACHE = _build()
    nc = _NC_CACHE

    Mk = np.ascontiguousarray(np.asarray(Mk), dtype=np.float32)
    Qk = np.ascontiguousarray(np.asarray(Qk), dtype=np.float32)

    in_maps = []
    for c in range(N_CORES):
        b, half = c // 2, c % 2
        mk = Mk[b].reshape(CK, N)
        qk = np.ascontiguousarray(Qk[b].reshape(CK, N)[:, half * QH:(half + 1) * QH])
        in_maps.append({"mk": mk, "qk": qk})

    res = run_bass_kernel_spmd(nc, in_maps, core_ids=list(range(N_CORES)))

    out = np.empty((B, N, N), dtype=np.float32)
    for c in range(N_CORES):
        b, half = c // 2, c % 2
        out[b, :, half * QH:(half + 1) * QH] = \
            np.asarray(res.results[c]["out"]).astype(np.float32).T
    return out
